# revision 2
# baseline (speedup 1.0000x reference)
"""MoE feed-forward (top-2 of 8 experts) on 8 Trainium2 NeuronCores.

Strategy (expert-parallel, host dispatch):
  - Host computes the (cheap) router: logits -> softmax -> top-2 -> renorm
    weights, plus the balance loss.  O(N*D*E) = 134 MFLOP of numpy work vs
    275 GFLOP of expert FFN work that goes to the device.
  - Tokens are gathered per expert and shipped (transposed, [D, C]) to the
    core that owns that expert.  Core e holds W1[e]/b1[e]/W2[e]/b2[e].
  - Each core computes   y^T = W2^T @ gelu(W1^T @ x^T + b1) + b2   for its
    tokens with both matmuls in float32r (full fp32 storage, 1 PE
    cycle/row), weights streamed from HBM in pre-packed DMA-friendly
    blocks, activations kept on-chip.
  - Host scatters y back with the routing weights and reshapes.

All shapes are hardcoded for B=4, T=2048, D=1024, E=8, K=2, H=4*D.
"""

import numpy as np

import concourse.bass as bass
from concourse import bacc, mybir, tile
from concourse.bass_utils import run_bass_kernel_spmd

B, T, D, E, TOPK = 4, 2048, 1024, 8, 2
H = 4 * D
N = B * T
N_CORES = 8
KT = D // 128   # 8   k-tiles (contraction d)
MT = H // 128   # 32  m-tiles (hidden h)
DT = D // 128   # 8   d-tiles (output d)
SUB = 512       # moving-operand sub-chunk (fp32 max 512)

F32 = mybir.dt.float32
F32R = mybir.dt.float32r
AF = mybir.ActivationFunctionType

_BUILD_CACHE = {}


def _build_nc(C, chunk=512):
    """Per-core expert FFN: yt[D, C] = W2^T @ gelu(W1^T @ xt + b1) + b2."""
    assert C % chunk == 0 and chunk % SUB == 0
    n_chunks = C // chunk
    subs = [(s, min(SUB, chunk - s)) for s in range(0, chunk, SUB)]

    nc = bacc.Bacc("TRN2", target_bir_lowering=False, debug=False,
                   num_devices=N_CORES)
    xt_d = nc.dram_tensor("xt", [D, C], F32R, kind="ExternalInput")
    w1_d = nc.dram_tensor("w1p", [MT, 128, KT * 128], F32R, kind="ExternalInput")
    b1_d = nc.dram_tensor("b1p", [128, MT], F32R, kind="ExternalInput")
    w2_d = nc.dram_tensor("w2p", [DT, 128, MT * 128], F32R, kind="ExternalInput")
    b2_d = nc.dram_tensor("b2p", [128, DT], F32R, kind="ExternalInput")
    yt_d = nc.dram_tensor("yt", [D, C], F32R, kind="ExternalOutput")

    with tile.TileContext(nc) as tc:
        with (
            tc.tile_pool(name="consts", bufs=1) as cpool,
            tc.tile_pool(name="xt", bufs=1) as xpool,
            tc.tile_pool(name="h", bufs=1) as hpool,
            tc.tile_pool(name="w1", bufs=3) as w1pool,
            tc.tile_pool(name="w2", bufs=2) as w2pool,
            tc.tile_pool(name="y", bufs=3) as ypool,
            tc.tile_pool(name="psA", bufs=2, space="PSUM") as psA,
            tc.tile_pool(name="psB", bufs=2, space="PSUM") as psB,
        ):
            b1_sb = cpool.tile([128, MT], F32R, tag="b1")
            nc.sync.dma_start(b1_sb[:], b1_d.ap())
            b2_sb = cpool.tile([128, DT], F32R, tag="b2")
            nc.sync.dma_start(b2_sb[:], b2_d.ap())

            for n in range(n_chunks):
                c0 = n * chunk
                # ---- load x^T chunk: 8 tiles [128(d), chunk] ----
                xts = []
                for k in range(KT):
                    t = xpool.tile([128, chunk], F32R, tag=f"xt{k}")
                    nc.sync.dma_start(
                        t[:], xt_d[k * 128:(k + 1) * 128, c0:c0 + chunk])
                    xts.append(t)

                # ---- phase A: h^T[m] = gelu(sum_k W1[k,m].T @ xt[k] + b1) ----
                hts = []
                for m in range(MT):
                    w1t = w1pool.tile([128, KT * 128], F32R, tag="w1")
                    nc.sync.dma_start(w1t[:], w1_d[m])
                    ht = hpool.tile([128, chunk], F32R, tag=f"h{m}")
                    for (s, w) in subs:
                        ph = psA.tile([128, SUB], F32, tag="psA")
                        for k in range(KT):
                            nc.tensor.matmul(
                                ph[:, :w],
                                w1t[:, k * 128:(k + 1) * 128],
                                xts[k][:, s:s + w],
                                start=(k == 0), stop=(k == KT - 1))
                        nc.scalar.activation(ht[:, s:s + w], ph[:, :w],
                                             AF.Gelu, bias=b1_sb[:, m:m + 1])
                    hts.append(ht)

                # ---- phase B: y^T[d] = sum_m W2[m,d].T @ h^T[m] + b2 ----
                for d in range(DT):
                    w2t = w2pool.tile([128, MT * 128], F32R, tag="w2")
                    nc.sync.dma_start(w2t[:], w2_d[d])
                    yt_sb = ypool.tile([128, chunk], F32R, tag="y")
                    for (s, w) in subs:
                        py = psB.tile([128, SUB], F32, tag="psB")
                        for m in range(MT):
                            nc.tensor.matmul(
                                py[:, :w],
                                w2t[:, m * 128:(m + 1) * 128],
                                hts[m][:, s:s + w],
                                start=(m == 0), stop=(m == MT - 1))
                        nc.scalar.activation(yt_sb[:, s:s + w], py[:, :w],
                                             AF.Identity, bias=b2_sb[:, d:d + 1])
                    nc.sync.dma_start(
                        yt_d[d * 128:(d + 1) * 128, c0:c0 + chunk], yt_sb[:])

    nc.compile()
    return nc


def _get_nc(C, chunk=512):
    key = (C, chunk)
    if key not in _BUILD_CACHE:
        _BUILD_CACHE[key] = _build_nc(C, chunk)
    return _BUILD_CACHE[key]


def _pack_w1(w1e):
    # w1p[m, p, k*128+c] = W1[k*128+p, m*128+c]
    return np.ascontiguousarray(
        w1e.reshape(KT, 128, MT, 128).transpose(2, 1, 0, 3).reshape(MT, 128, KT * 128))


def _pack_w2(w2e):
    # w2p[d, p, m*128+c] = W2[m*128+p, d*128+c]
    return np.ascontiguousarray(
        w2e.reshape(MT, 128, DT, 128).transpose(2, 1, 0, 3).reshape(DT, 128, MT * 128))


def _route(x_flat, latent_code, Wr, Wlr):
    """Numpy replica of the reference router (fp32)."""
    logits = (x_flat @ Wr).reshape(B, T, E) + (latent_code @ Wlr)[:, None, :]
    logits = logits.reshape(N, E)
    mx = logits.max(axis=1, keepdims=True)
    p = np.exp(logits - mx)
    probs = (p / p.sum(axis=1, keepdims=True)).astype(np.float32)
    sel = np.argsort(-probs, axis=1, kind="stable")[:, :TOPK]
    top_w = np.take_along_axis(probs, sel, axis=1)
    top_w = top_w / top_w.sum(axis=1, keepdims=True)
    # balance loss
    prob_mean = probs.mean(axis=0)
    frac = np.zeros(E, dtype=np.float64)
    for j in range(TOPK):
        frac += np.bincount(sel[:, j], minlength=E)
    frac_mean = (frac / N).astype(np.float32)
    balance_loss = np.float32(E * np.sum(prob_mean * frac_mean, dtype=np.float32))
    return sel, top_w.astype(np.float32), balance_loss


def run(inputs, trace=False, chunk=512):
    x = np.asarray(inputs["x"], dtype=np.float32)
    latent_code = np.asarray(inputs["latent_code"], dtype=np.float32)
    Wr = np.asarray(inputs["Wr"], dtype=np.float32)
    Wlr = np.asarray(inputs["Wlr"], dtype=np.float32)
    W1 = np.asarray(inputs["W1"], dtype=np.float32)
    b1 = np.asarray(inputs["b1"], dtype=np.float32)
    W2 = np.asarray(inputs["W2"], dtype=np.float32)
    b2 = np.asarray(inputs["b2"], dtype=np.float32)

    x_flat = x.reshape(N, D)
    sel, top_w, balance_loss = _route(x_flat, latent_code, Wr, Wlr)

    # per-expert token lists
    idxs, wts = [], []
    for e in range(E):
        hit = (sel == e)
        tok = np.where(hit.any(axis=1))[0]
        w = np.where(hit[tok, 0], top_w[tok, 0], top_w[tok, 1])
        idxs.append(tok)
        wts.append(w.astype(np.float32))
    max_cnt = max(len(t) for t in idxs)
    C = -(-max_cnt // chunk) * chunk

    nc = _get_nc(C, chunk)

    in_maps = []
    for e in range(E):
        tok = idxs[e]
        xt = np.zeros((D, C), dtype=np.float32)
        xt[:, :len(tok)] = x_flat[tok].T
        in_maps.append({
            "xt": xt,
            "w1p": _pack_w1(W1[e]),
            "b1p": np.ascontiguousarray(b1[e].reshape(MT, 128).T),
            "w2p": _pack_w2(W2[e]),
            "b2p": np.ascontiguousarray(b2[e].reshape(DT, 128).T),
        })

    res = run_bass_kernel_spmd(nc, in_maps, core_ids=list(range(N_CORES)),
                               trace=trace)

    out_flat = np.zeros((N, D), dtype=np.float32)
    for e in range(E):
        tok = idxs[e]
        yt = res.results[e]["yt"]
        out_flat[tok] += wts[e][:, None] * yt[:, :len(tok)].T

    return (out_flat.reshape(B, T, D), balance_loss), res


def kernel(**inputs):
    (out, balance_loss), _ = run(inputs, trace=False)
    return out, balance_loss
